# revision 41
# baseline (speedup 1.0000x reference)
"""Channel-attention ("attention transpose") Trainium2 kernel, v3.

Reference computation (per batch b):
    attn = softmax(scale * x1[b].T @ x2[b], axis=-1)   # [C, C]
    out[b] = attn @ x2[b].T                            # [C, N]

Sharding: data-parallel over batch B=8 across the 8 NeuronCores; each core
computes one batch independently (no collectives).

Design: one serial DMA queue ordered so PE never starves.
  - x2 chunks stream by DIRECT DMA into a resident fp32r image (bitcast
    view, no staging); x1 c-blocks stream the same way (fp32r, sub-block
    granularity) staggered into the x2 stream.
  - G=3 logits groups (c0..c2, fp32r x fp32r) chase the stream (PSUM 6
    banks), with c1/c2 starting at chunk offsets 2/6 and wrapping around
    over the resident image; wrap matmuls fill the under-utilized early
    positions. Per-chunk transposes (Pool bf16 cast -> PE identity matmul)
    also run during the stream, building a full x2T [d,n] bf16 image, so
    phase 2 needs no further transposes or casts.
  - Post-stream: per-c unnormalized softmax (no max subtraction: scaled
    logits are far below exp overflow) -> attnT transpose -> out rows,
    interleaved with the remaining 3 c-tiles' logits which chase the x1
    c3..c5 loads.
"""

import numpy as np

import concourse.bass as bass
import concourse.mybir as mybir
import concourse.tile as tile
from concourse import bacc
from concourse.bass_utils import run_bass_kernel_spmd
from concourse.masks import make_identity

B, N, C = 8, 4096, 768
SCALE = (C // 8) ** -0.5  # 96^-0.5
P = 128
NO = N // P       # 32 n-chunks
CT = C // P       # 6 c-tiles
DH = C // 2       # 384, d-half free width for logits matmuls
NPAN = 8          # phase-2 n panels
PAN = N // NPAN   # 512
SUB = 8           # n-chunks per x1 sub-load
NSUB = NO // SUB  # 4 subs per x1 block

CONFIG = {
    "starts": (0, 1, 3),   # chunk start offset per g0 c-tile
    # x1 sub-loads issued before chunk <key> in the DMA queue; sub q of
    # block c must precede the first chunk position that uses it
    "x1_slots": {0: [(0, 0)], 1: [(1, 0)], 2: [(2, 0)],
                 **{8 * q: [(0, q), (1, q), (2, q)] for q in range(1, 4)}},
    # x2 chunk-group sizes: fine-grained front for the chase, double after
    "x2_groups": [(0, 1), (1, 1)] + [(c, 2) for c in range(2, 32, 2)],
    # wrap-around chunk matmuls emitted at chunk <key>'s position
    "wraps": {2: [(1, 0)], 4: [(2, 0)], 5: [(2, 1)], 6: [(2, 2)]},
}

F32 = mybir.dt.float32
F32R = mybir.dt.float32r
BF16 = mybir.dt.bfloat16
AX = mybir.AxisListType
AF = mybir.ActivationFunctionType


def stream_schedule():
    sched = []
    for (ch0, n) in CONFIG["x2_groups"]:
        for ch in range(ch0, ch0 + n):
            for (c, q) in CONFIG["x1_slots"].get(ch, ()):
                sched.append(("x1", c, q))
        sched.append(("x2", ch0, n))
    return sched


def build_body(nc, tc, pools, identity_bf, x1, x2, out, variant="full"):
    (singles, x1st_pool, x1stb_pool, x1blk_pool, x2st_pool, attnt_pool,
     attnc_pool, opool, stats, ps_attn, ps_misc) = pools

    x2_sb = singles.tile([P, NO, C], BF16, tag="x2_sb")
    x2T = singles.tile([P, CT, N], BF16, tag="x2T")
    recip = singles.tile([P, CT], F32, tag="recip")

    x2_t = x2.rearrange("(no p) d -> p no d", p=P)
    x1_t = x1.rearrange("(no p) c -> p no c", p=P)

    x1_blocks = {}
    ps_tiles = {}
    starts = {i: s for i, s in enumerate(CONFIG["starts"])}

    def issue_x1_sub(c, q):
        # staged f32 -> DVE cast to bf16 (DVE, not Pool: Pool is a dedicated
        # low-jitter x2-cast lane during the stream)
        if c not in x1_blocks:
            x1_blocks[c] = x1blk_pool.tile([P, NO, P], BF16, tag="x1_blk",
                                           name=f"x1b{c}")
        st = x1st_pool.tile([P, SUB, P], F32, tag="x1_st", name=f"x1s{c}_{q}")
        nc.sync.dma_start(out=st, in_=x1_t[:, q * SUB:(q + 1) * SUB,
                                          c * P:(c + 1) * P])
        nc.vector.tensor_copy(out=x1_blocks[c][:, q * SUB:(q + 1) * SUB, :],
                              in_=st)

    def logits_mm(c, ch, first, last):
        ps = ps_tiles[c]
        lhsT = x1_blocks[c][:, ch, :]
        nc.tensor.matmul(ps[:, 0, :DH], lhsT, x2_sb[:, ch, 0:DH],
                         start=first, stop=last)
        nc.tensor.matmul(ps[:, 1, :DH], lhsT, x2_sb[:, ch, DH:2 * DH],
                         start=first, stop=last)

    def chunk_transposes(ch):
        # PE transposes the resident bf16 chunk via identity matmuls (two
        # PSUM tiles: 4 then 2 d-tiles); copy-outs split DVE/ACT
        xb = x2_sb[:, ch, :]
        for db, nd in ((0, 4), (4, 2)):
            pt = ps_misc.tile([P, 512], F32, tag="ps_misc",
                              name=f"ptS{ch}_{db}")
            for j in range(nd):
                d = db + j
                nc.tensor.matmul(pt[:, j * P:(j + 1) * P],
                                 xb[:, d * P:(d + 1) * P],
                                 identity_bf, start=True, stop=True)
            dst = x2T[:, db:db + nd, ch * P:(ch + 1) * P]
            src_v = pt[:, :nd * P].rearrange("p (j q) -> p j q", j=nd)
            if db == 0:
                nc.vector.tensor_copy(out=dst, in_=src_v)
            else:
                nc.scalar.copy(out=dst, in_=src_v)

    # ---- Stream phase ----
    for item in stream_schedule():
        if item[0] == "x1":
            issue_x1_sub(item[1], item[2])
            continue
        ch0, ng = item[1], item[2]
        st = x2st_pool.tile([P, 2, C], F32, tag="x2_st", name=f"x2s{ch0}")
        if ch0 == 0:
            # split the very first chunk in d-halves so logits start earlier
            for h in range(2):
                sl = slice(h * DH, (h + 1) * DH)
                nc.sync.dma_start(out=st[:, 0, sl], in_=x2_t[:, 0, sl])
                nc.gpsimd.tensor_copy(out=x2_sb[:, 0, sl], in_=st[:, 0, sl])
        else:
            nc.sync.dma_start(out=st[:, :ng, :],
                              in_=x2_t[:, ch0:ch0 + ng, :])
            nc.gpsimd.tensor_copy(out=x2_sb[:, ch0:ch0 + ng, :],
                                  in_=st[:, :ng, :])
        for ch in range(ch0, ch0 + ng):
            for c, s in starts.items():
                if c not in ps_tiles:
                    ps_tiles[c] = ps_attn.tile([P, 2, 512], F32,
                                               tag="ps_attn", name=f"ps{c}")
                if ch >= s:
                    logits_mm(c, ch, first=(ch == s), last=(ch == NO - 1))
            for (wc, w) in CONFIG["wraps"].get(ch, ()):
                logits_mm(wc, w, first=False, last=False)
            chunk_transposes(ch)

    # ---- x1 c3..c5 loads (after the x2 stream): one whole-block DMA each
    for c in range(3, CT):
        x1_blocks[c] = x1blk_pool.tile([P, NO, P], BF16, tag="x1_blk",
                                       name=f"x1b{c}")
        stb = x1stb_pool.tile([P, NO, P], F32, tag="x1_stB", name=f"x1B{c}")
        nc.sync.dma_start(out=stb, in_=x1_t[:, :, c * P:(c + 1) * P])
        nc.vector.tensor_copy(out=x1_blocks[c], in_=stb)

    def softmax(c):
        # Unnormalized softmax: scaled logits are bounded far below f32 exp
        # overflow, so no max-subtraction pass is needed.
        ps = ps_tiles[c]
        ssum = stats.tile([P, 2], F32, tag="ssum", name=f"ss{c}")
        attn_c = attnc_pool.tile([P, C], BF16, tag="attn_c", name=f"ac{c}")
        for h in range(2):
            nc.scalar.activation(
                out=attn_c[:, h * DH:(h + 1) * DH],
                in_=ps[:, h, :DH],
                func=AF.Exp,
                bias=0.0,
                scale=SCALE,
                accum_out=ssum[:, h:h + 1],
            )
        stot = stats.tile([P, 1], F32, tag="stot", name=f"st{c}")
        nc.vector.reduce_sum(out=stot, in_=ssum, axis=AX.X)
        nc.vector.reciprocal(out=recip[:, c:c + 1], in_=stot)
        return attn_c

    def attnt_tr(c, attn_c):
        at = attnt_pool.tile([P, CT, P], BF16, tag="attnT", name=f"at{c}")
        if CONFIG.get("attnt_xbar"):
            # XBAR DMA transpose: frees PE/PSUM in the sim's cost model but
            # measured ~5x slower per op on real HW -- keep disabled
            nc.sync.dma_start_transpose(out=at, in_=attn_c)
            return at
        for db, nd in ((0, 4), (4, 2)):
            pt = ps_misc.tile([P, 512], F32, tag="ps_misc",
                              name=f"ptA{c}_{db}")
            for j in range(nd):
                d = db + j
                nc.tensor.matmul(pt[:, j * P:(j + 1) * P],
                                 attn_c[:, d * P:(d + 1) * P],
                                 identity_bf, start=True, stop=True)
            dst = at[:, db:db + nd, :]
            src_v = pt[:, :nd * P].rearrange("p (j q) -> p j q", j=nd)
            if db == 0:
                nc.vector.tensor_copy(out=dst, in_=src_v)
            else:
                nc.scalar.copy(out=dst, in_=src_v)
        return at

    def po_block(c, at, last=False):
        for pp in range(NPAN // 2):
            ob = opool.tile([P, 2, PAN], F32, tag="ob", name=f"ob{c}_{pp}")
            for h in range(2):
                pan = 2 * pp + h
                po = ps_misc.tile([P, PAN], F32, tag="ps_misc",
                                  name=f"po{c}_{pan}")
                for d in range(CT):
                    nc.tensor.matmul(po, at[:, d, :],
                                     x2T[:, d, pan * PAN:(pan + 1) * PAN],
                                     start=(d == 0), stop=(d == CT - 1))
                nc.scalar.activation(out=ob[:, h, :], in_=po, func=AF.Copy,
                                     scale=recip[:, c:c + 1])
            nc.sync.dma_start(
                out=out[c * P:(c + 1) * P,
                        pp * 2 * PAN:(pp + 1) * 2 * PAN],
                in_=ob.rearrange("p a b -> p (a b)"))

    def logits_block(c):
        ps_tiles[c] = ps_attn.tile([P, 2, 512], F32, tag="ps_attn",
                                   name=f"ps{c}")
        for ch in range(NO):
            logits_mm(c, ch, first=(ch == 0), last=(ch == NO - 1))

    # ---- Post-stream ----
    ac = {}
    ac[0] = softmax(0)
    ac[1] = softmax(1)
    ac[2] = softmax(2)
    at0 = attnt_tr(0, ac[0])
    po_block(0, at0)
    logits_block(3)
    ac[3] = softmax(3)
    at1 = attnt_tr(1, ac[1])
    po_block(1, at1)
    logits_block(4)
    ac[4] = softmax(4)
    at3 = attnt_tr(3, ac[3])
    at2 = attnt_tr(2, ac[2])
    po_block(2, at2)
    logits_block(5)
    ac[5] = softmax(5)
    at4 = attnt_tr(4, ac[4])
    po_block(3, at3)
    at5 = attnt_tr(5, ac[5])
    po_block(4, at4)
    po_block(5, at5, last=True)


def build_kernel(reps=1, variant="full"):
    nc = bacc.Bacc("TRN2", target_bir_lowering=False, debug=False,
                   num_devices=8)
    x1 = nc.declare_dram_parameter("x_1", [N, C], F32, isOutput=False)
    x2 = nc.declare_dram_parameter("x_2", [N, C], F32, isOutput=False)
    out = nc.declare_dram_parameter("out", [C, N], F32, isOutput=True)

    with tile.TileContext(nc) as tc:
        with (
            tc.tile_pool(name="singles", bufs=1) as singles,
            tc.tile_pool(name="x1st", bufs=4) as x1st_pool,
            tc.tile_pool(name="x1stb", bufs=1) as x1stb_pool,
            tc.tile_pool(name="x1blk", bufs=3) as x1blk_pool,
            tc.tile_pool(name="x2st", bufs=5) as x2st_pool,
            tc.tile_pool(name="attnt", bufs=3) as attnt_pool,
            tc.tile_pool(name="attnc", bufs=2) as attnc_pool,
            tc.tile_pool(name="opool", bufs=2) as opool,
            tc.tile_pool(name="stats", bufs=4) as stats,
            tc.tile_pool(name="ps_attn", bufs=3, space="PSUM") as ps_attn,
            tc.tile_pool(name="ps_misc", bufs=2, space="PSUM") as ps_misc,
        ):
            pools = (singles, x1st_pool, x1stb_pool, x1blk_pool, x2st_pool,
                     attnt_pool, attnc_pool, opool, stats, ps_attn, ps_misc)
            identity_bf = singles.tile([P, P], BF16, tag="identity_bf")
            make_identity(nc, identity_bf)
            for _ in range(reps):
                build_body(nc, tc, pools, identity_bf, x1[:], x2[:], out[:],
                           variant=variant)
    nc.compile()
    return nc


_nc_cache = {}


def get_kernel(reps=1, variant="full"):
    key = (reps, variant)
    if key not in _nc_cache:
        _nc_cache[key] = build_kernel(reps, variant)
    return _nc_cache[key]


def kernel(x_1, x_2):
    x_1 = np.asarray(x_1, dtype=np.float32)
    x_2 = np.asarray(x_2, dtype=np.float32)
    assert x_1.shape == (B, N, C) and x_2.shape == (B, N, C)
    nc = get_kernel(reps=1)
    core_ids = list(range(8))
    in_maps = [
        {"x_1": np.ascontiguousarray(x_1[b]),
         "x_2": np.ascontiguousarray(x_2[b])}
        for b in core_ids
    ]
    res = run_bass_kernel_spmd(nc, in_maps, core_ids)
    return np.stack([res.results[b]["out"] for b in core_ids], axis=0)
